# revision 1
# baseline (speedup 1.0000x reference)
"""3x3 median blur (replicate padding) on Trainium2, 8-core data parallel.

Problem: noised_image [32,3,512,512] f32 -> median-blurred; cover_image passthrough.

Strategy:
- Shard batch across 8 NeuronCores: 4 images (12 channel-planes) per core.
- Host-side edge-pad each 512x512 plane to 514x514 so the device kernel needs
  exactly one input DMA per plane (no replicate logic on device).
- Per plane ("strip"): partition p holds padded rows 4p..4p+5 (6 rows x 514 f32
  in the free dim), so every tap of the 3x3 window is a free-dim AP offset.
- Exact median-of-9 via an 18-op min/max network, all on the Vector engine
  (this toolchain's GPSIMD lacks TensorTensor; ACT/PE/DMA-CCE can't do f32
  min/max either): vertical sort3 per column (pairs pmn/pmx, then lo/mid/hi),
  then the horizontal combine med3(max3(lo), med3(mid), min3(hi)) with
  sliding-window reuse. All f32 min/max -> bit-exact vs jnp.sort(...)[..., 4].
  The 8-op combine tail is proven minimal by exhaustive lattice search.
- Raw Bass program (explicit semaphores, standalone wait_ge sequencer
  instructions), double-buffered input/output tiles, DMA on the sync (SP)
  engine overlapping compute.
"""
import sys
sys.path.insert(0, '/opt/trn_rl_repo')
from contextlib import ExitStack
import numpy as np

import concourse.bass as bass
import concourse.mybir as mybir
import bass_rust
from concourse import bass_utils

F32 = mybir.dt.float32
MIN = mybir.AluOpType.min
MAX = mybir.AluOpType.max

N_CORES = 8
N_CH = 12          # channel-planes per core (4 images x 3 channels)
H = W = 512
HP = WP = 514      # host-padded plane
R = 4              # output rows per partition


def _mk_ap(base, dims, offset):
    c = base.copy()
    c.ap = bass_rust.VecI64Pair(dims)
    c.offset = offset
    return c


def _build_nc(n_ch=N_CH, reps=1, use_gpsimd=False):
    nc = bass.Bass("TRN2")
    x = nc.dram_tensor("x", [n_ch, HP, WP], F32, kind="ExternalInput")
    y = nc.dram_tensor("y", [n_ch, W, W], F32, kind="ExternalOutput")
    DOPS = 13 if use_gpsimd else 18
    GOPS = 5
    with ExitStack() as ctx:
        xs = [ctx.enter_context(nc.sbuf_tensor(f"xs{i}", [128, 6, WP], F32)) for i in range(2)]
        out = [ctx.enter_context(nc.sbuf_tensor(f"outb{i}", [128, R, W], F32)) for i in range(2)]
        pmn = ctx.enter_context(nc.sbuf_tensor("pmn", [128, 5, WP], F32))
        pmx = ctx.enter_context(nc.sbuf_tensor("pmx", [128, 5, WP], F32))
        lo3 = ctx.enter_context(nc.sbuf_tensor("lo3", [128, R, WP], F32))
        hi3 = ctx.enter_context(nc.sbuf_tensor("hi3", [128, R, WP], F32))
        mid3 = ctx.enter_context(nc.sbuf_tensor("mid3", [128, R, WP], F32))
        mlo = ctx.enter_context(nc.sbuf_tensor("mlo", [128, R, W + 1], F32))
        mhi = ctx.enter_context(nc.sbuf_tensor("mhi", [128, R, W + 1], F32))
        qmn = ctx.enter_context(nc.sbuf_tensor("qmn", [128, R, W + 1], F32))
        qmx = ctx.enter_context(nc.sbuf_tensor("qmx", [128, R, W + 1], F32))
        A = ctx.enter_context(nc.sbuf_tensor("A", [128, R, W], F32))
        u = ctx.enter_context(nc.sbuf_tensor("u", [128, R, W], F32))
        B = ctx.enter_context(nc.sbuf_tensor("B", [128, R, W], F32))
        fmn = ctx.enter_context(nc.sbuf_tensor("fmn", [128, R, W], F32))
        fmx = ctx.enter_context(nc.sbuf_tensor("fmx", [128, R, W], F32))
        v = ctx.enter_context(nc.sbuf_tensor("v", [128, R, W], F32))
        if use_gpsimd:
            ttb = [ctx.enter_context(nc.sbuf_tensor(f"ttb{i}", [128, R, WP], F32)) for i in range(2)]
            Cb = [ctx.enter_context(nc.sbuf_tensor(f"Cb{i}", [128, R, W], F32)) for i in range(2)]
        else:
            tt = ctx.enter_context(nc.sbuf_tensor("tt", [128, R, WP], F32))
            C = ctx.enter_context(nc.sbuf_tensor("C", [128, R, W], F32))

        sem_in = ctx.enter_context(nc.semaphore())
        sem_out = ctx.enter_context(nc.semaphore())
        sem_dve = ctx.enter_context(nc.semaphore())
        sem_gp = ctx.enter_context(nc.semaphore())

        block = ctx.enter_context(nc.Block())
        n_strips = n_ch * reps

        @block.sync
        def _(sync):
            for i in range(n_strips):
                ch = i % n_ch
                if i >= 2:
                    sync.wait_ge(sem_dve, DOPS * (i - 2) + 2)
                    if use_gpsimd:
                        sync.wait_ge(sem_gp, GOPS * (i - 2) + 3)
                src = _mk_ap(x[ch], [[R * WP, 128], [WP, 6], [1, WP]], ch * HP * WP)
                sync.dma_start(xs[i % 2][:, :, :], src).then_inc(sem_in, 16)
                if i >= 1:
                    oi = i - 1
                    sync.wait_ge(sem_dve, DOPS * (oi + 1))
                    dst = y[oi % n_ch].rearrange("(p r) w -> p r w", r=R)
                    sync.dma_start(dst, out[oi % 2][:, :, :]).then_inc(sem_out, 16)
            oi = n_strips - 1
            sync.wait_ge(sem_dve, DOPS * (oi + 1))
            dst = y[oi % n_ch].rearrange("(p r) w -> p r w", r=R)
            sync.dma_start(dst, out[oi % 2][:, :, :]).then_inc(sem_out, 16)

        if use_gpsimd:
            @block.gpsimd
            def _(gp):
                for i in range(n_strips):
                    xv = xs[i % 2]
                    tv = ttb[i % 2]
                    Cv = Cb[i % 2]
                    gp.wait_ge(sem_in, 16 * (i + 1))
                    t = gp.tensor_tensor(pmx[:, :, :], xv[:, 0:5, :], xv[:, 1:6, :], MAX); t.then_inc(sem_gp, 1)
                    t = gp.tensor_tensor(hi3[:, :, :], pmx[:, 0:R, :], xv[:, 2:6, :], MAX); t.then_inc(sem_gp, 1)
                    if i >= 2:
                        gp.wait_ge(sem_dve, DOPS * (i - 2) + 3)
                    t = gp.tensor_tensor(tv[:, :, :], pmx[:, 0:R, :], xv[:, 2:6, :], MIN); t.then_inc(sem_gp, 1)
                    t = gp.tensor_tensor(mhi[:, :, :], hi3[:, :, 0:W + 1], hi3[:, :, 1:WP], MIN); t.then_inc(sem_gp, 1)
                    if i >= 2:
                        gp.wait_ge(sem_dve, DOPS * (i - 2) + 12)
                    t = gp.tensor_tensor(Cv[:, :, :], mhi[:, :, 0:W], hi3[:, :, 2:WP], MIN); t.then_inc(sem_gp, 1)

        @block.vector
        def _(vector):
            for i in range(n_strips):
                xv = xs[i % 2]
                ov = out[i % 2]
                if use_gpsimd:
                    tv = ttb[i % 2]
                    Cv = Cb[i % 2]
                vector.wait_ge(sem_in, 16 * (i + 1))
                t = vector.tensor_tensor(pmn[:, :, :], xv[:, 0:5, :], xv[:, 1:6, :], MIN); t.then_inc(sem_dve, 1)
                if not use_gpsimd:
                    t = vector.tensor_tensor(pmx[:, :, :], xv[:, 0:5, :], xv[:, 1:6, :], MAX); t.then_inc(sem_dve, 1)
                t = vector.tensor_tensor(lo3[:, :, :], pmn[:, 0:R, :], xv[:, 2:6, :], MIN); t.then_inc(sem_dve, 1)
                if not use_gpsimd:
                    t = vector.tensor_tensor(hi3[:, :, :], pmx[:, 0:R, :], xv[:, 2:6, :], MAX); t.then_inc(sem_dve, 1)
                    t = vector.tensor_tensor(tt[:, :, :], pmx[:, 0:R, :], xv[:, 2:6, :], MIN); t.then_inc(sem_dve, 1)
                    tv = tt
                else:
                    vector.wait_ge(sem_gp, GOPS * i + 3)
                t = vector.tensor_tensor(mid3[:, :, :], pmn[:, 0:R, :], tv[:, :, :], MAX); t.then_inc(sem_dve, 1)
                t = vector.tensor_tensor(mlo[:, :, :], lo3[:, :, 0:W + 1], lo3[:, :, 1:WP], MAX); t.then_inc(sem_dve, 1)
                if not use_gpsimd:
                    t = vector.tensor_tensor(mhi[:, :, :], hi3[:, :, 0:W + 1], hi3[:, :, 1:WP], MIN); t.then_inc(sem_dve, 1)
                t = vector.tensor_tensor(qmn[:, :, :], mid3[:, :, 0:W + 1], mid3[:, :, 1:WP], MIN); t.then_inc(sem_dve, 1)
                t = vector.tensor_tensor(qmx[:, :, :], mid3[:, :, 0:W + 1], mid3[:, :, 1:WP], MAX); t.then_inc(sem_dve, 1)
                t = vector.tensor_tensor(A[:, :, :], mlo[:, :, 0:W], lo3[:, :, 2:WP], MAX); t.then_inc(sem_dve, 1)
                if not use_gpsimd:
                    t = vector.tensor_tensor(C[:, :, :], mhi[:, :, 0:W], hi3[:, :, 2:WP], MIN); t.then_inc(sem_dve, 1)
                    Cv = C
                t = vector.tensor_tensor(u[:, :, :], qmx[:, :, 0:W], mid3[:, :, 2:WP], MIN); t.then_inc(sem_dve, 1)
                t = vector.tensor_tensor(B[:, :, :], qmn[:, :, 0:W], u[:, :, :], MAX); t.then_inc(sem_dve, 1)
                t = vector.tensor_tensor(fmn[:, :, :], A[:, :, :], B[:, :, :], MIN); t.then_inc(sem_dve, 1)
                t = vector.tensor_tensor(fmx[:, :, :], A[:, :, :], B[:, :, :], MAX); t.then_inc(sem_dve, 1)
                if use_gpsimd:
                    vector.wait_ge(sem_gp, GOPS * i + 5)
                t = vector.tensor_tensor(v[:, :, :], fmx[:, :, :], Cv[:, :, :], MIN); t.then_inc(sem_dve, 1)
                if i >= 2:
                    vector.wait_ge(sem_out, 16 * (i - 1))
                t = vector.tensor_tensor(ov[:, :, :], fmn[:, :, :], v[:, :, :], MAX); t.then_inc(sem_dve, 1)
    return nc


_NC_CACHE = {}


def _get_nc(use_gpsimd=False):
    key = use_gpsimd
    if key not in _NC_CACHE:
        _NC_CACHE[key] = _build_nc(use_gpsimd=use_gpsimd)
    return _NC_CACHE[key]


def kernel(noised_image, cover_image):
    noised_image = np.ascontiguousarray(noised_image, dtype=np.float32)
    nc = _get_nc(use_gpsimd=False)
    per = noised_image.shape[0] // N_CORES  # 4 images per core
    in_maps = []
    for c in range(N_CORES):
        shard = noised_image[c * per:(c + 1) * per].reshape(N_CH, H, W)
        padded = np.pad(shard, ((0, 0), (1, 1), (1, 1)), mode='edge')
        in_maps.append({"x": np.ascontiguousarray(padded)})
    res = bass_utils.run_bass_kernel_spmd(nc, in_maps, core_ids=list(range(N_CORES)))
    blurred = np.stack([r["y"].reshape(per, 3, H, W) for r in res.results])
    blurred = blurred.reshape(noised_image.shape).astype(np.float32)
    return (blurred, cover_image)



# revision 2
# speedup vs baseline: 2.4898x; 2.4898x over previous
"""3x3 median blur (replicate padding) on Trainium2, 8-core data parallel, fp16.

Problem: noised_image [32,3,512,512] f32 -> median-blurred; cover_image passthrough.

Strategy:
- Shard batch across 8 NeuronCores: 4 images (12 channel-planes) per core.
- Host-side: convert to fp16 (median commutes with monotone rounding, so the
  result equals the fp16-rounded exact median; rel err ~5e-4 << 2e-2 gate) and
  edge-pad each 512x512 plane to 514x514.
- fp16 doubles DVE tensor_tensor throughput (measured ~2.4-2.7 elem/cyc/partition
  vs ~1.1 for f32 on this HW; odd-element AP offsets do NOT drop the fast mode).
- Per strip = TWO planes: partition p holds padded rows 8q..8q+9 of plane
  2s + p//64 (q = p%64), i.e. R=8 output rows per partition. Fewer, larger ops
  amortize per-instruction overhead; 10/8 input overread (vs 6/4 at R=4).
- Exact median-of-9 via the 18-op min/max network: vertical sort3 per column
  (pairs pmn/pmx then lo/mid/hi), horizontal combine
  med3(max3(lo), med3(mid), min3(hi)) with sliding-window reuse.
- Raw Bass program, double-buffered input/output tiles, DMA on the sync (SP)
  engine overlapping compute; two DMAs per strip (one per plane half).
"""
import sys
sys.path.insert(0, '/opt/trn_rl_repo')
from contextlib import ExitStack
import numpy as np

import concourse.bass as bass
import concourse.mybir as mybir
import bass_rust
from concourse import bass_utils

F16 = mybir.dt.float16
MIN = mybir.AluOpType.min
MAX = mybir.AluOpType.max

N_CORES = 8
N_CH = 12          # channel-planes per core (4 images x 3 channels)
N_STRIPS = 6       # 2 planes per strip
H = W = 512
HP = WP = 514      # host-padded plane
R = 8              # output rows per partition (2 planes x 512 rows / 128)


def _mk_ap(base, dims, offset):
    c = base.copy()
    c.ap = bass_rust.VecI64Pair(dims)
    c.offset = offset
    return c


def _build_nc(reps=1):
    nc = bass.Bass("TRN2")
    x = nc.dram_tensor("x", [N_CH, HP, WP], F16, kind="ExternalInput")
    y = nc.dram_tensor("y", [N_CH, W, W], F16, kind="ExternalOutput")
    DOPS = 18
    with ExitStack() as ctx:
        xs = [ctx.enter_context(nc.sbuf_tensor(f"xs{i}", [128, R + 2, WP], F16)) for i in range(2)]
        out = [ctx.enter_context(nc.sbuf_tensor(f"outb{i}", [128, R, W], F16)) for i in range(2)]
        pmn = ctx.enter_context(nc.sbuf_tensor("pmn", [128, R + 1, WP], F16))
        pmx = ctx.enter_context(nc.sbuf_tensor("pmx", [128, R + 1, WP], F16))
        lo3 = ctx.enter_context(nc.sbuf_tensor("lo3", [128, R, WP], F16))
        hi3 = ctx.enter_context(nc.sbuf_tensor("hi3", [128, R, WP], F16))
        tt = ctx.enter_context(nc.sbuf_tensor("tt", [128, R, WP], F16))
        mid3 = ctx.enter_context(nc.sbuf_tensor("mid3", [128, R, WP], F16))
        mlo = ctx.enter_context(nc.sbuf_tensor("mlo", [128, R, W + 1], F16))
        mhi = ctx.enter_context(nc.sbuf_tensor("mhi", [128, R, W + 1], F16))
        qmn = ctx.enter_context(nc.sbuf_tensor("qmn", [128, R, W + 1], F16))
        qmx = ctx.enter_context(nc.sbuf_tensor("qmx", [128, R, W + 1], F16))
        A = ctx.enter_context(nc.sbuf_tensor("A", [128, R, W], F16))
        C = ctx.enter_context(nc.sbuf_tensor("C", [128, R, W], F16))
        u = ctx.enter_context(nc.sbuf_tensor("u", [128, R, W], F16))
        B = ctx.enter_context(nc.sbuf_tensor("B", [128, R, W], F16))
        fmn = ctx.enter_context(nc.sbuf_tensor("fmn", [128, R, W], F16))
        fmx = ctx.enter_context(nc.sbuf_tensor("fmx", [128, R, W], F16))
        v = ctx.enter_context(nc.sbuf_tensor("v", [128, R, W], F16))

        sem_in = ctx.enter_context(nc.semaphore())
        sem_out = ctx.enter_context(nc.semaphore())
        sem_dve = ctx.enter_context(nc.semaphore())

        block = ctx.enter_context(nc.Block())
        n_strips = N_STRIPS * reps

        @block.sync
        def _(sync):
            for i in range(n_strips):
                s = i % N_STRIPS
                if i >= 2:
                    # xs[i%2] free once strip i-2's last read of it (op 5: tt) is done
                    sync.wait_ge(sem_dve, DOPS * (i - 2) + 5)
                for h in range(2):
                    pl = 2 * s + h
                    src = _mk_ap(x[0], [[R * WP, 64], [WP, R + 2], [1, WP]],
                                 pl * HP * WP)
                    sync.dma_start(xs[i % 2][64 * h:64 * (h + 1), :, :], src).then_inc(sem_in, 16)
                if i >= 1:
                    oi = i - 1
                    sync.wait_ge(sem_dve, DOPS * (oi + 1))
                    os_ = oi % N_STRIPS
                    for h in range(2):
                        pl = 2 * os_ + h
                        dst = _mk_ap(y[0], [[R * W, 64], [W, R], [1, W]],
                                     pl * W * W)
                        sync.dma_start(dst, out[oi % 2][64 * h:64 * (h + 1), :, :]).then_inc(sem_out, 16)
            oi = n_strips - 1
            sync.wait_ge(sem_dve, DOPS * (oi + 1))
            os_ = oi % N_STRIPS
            for h in range(2):
                pl = 2 * os_ + h
                dst = _mk_ap(y[0], [[R * W, 64], [W, R], [1, W]], pl * W * W)
                sync.dma_start(dst, out[oi % 2][64 * h:64 * (h + 1), :, :]).then_inc(sem_out, 16)

        @block.vector
        def _(vector):
            for i in range(n_strips):
                xv = xs[i % 2]
                ov = out[i % 2]
                vector.wait_ge(sem_in, 32 * (i + 1))
                t = vector.tensor_tensor(pmn[:, :, :], xv[:, 0:R + 1, :], xv[:, 1:R + 2, :], MIN); t.then_inc(sem_dve, 1)
                t = vector.tensor_tensor(pmx[:, :, :], xv[:, 0:R + 1, :], xv[:, 1:R + 2, :], MAX); t.then_inc(sem_dve, 1)
                t = vector.tensor_tensor(lo3[:, :, :], pmn[:, 0:R, :], xv[:, 2:R + 2, :], MIN); t.then_inc(sem_dve, 1)
                t = vector.tensor_tensor(hi3[:, :, :], pmx[:, 0:R, :], xv[:, 2:R + 2, :], MAX); t.then_inc(sem_dve, 1)
                t = vector.tensor_tensor(tt[:, :, :], pmx[:, 0:R, :], xv[:, 2:R + 2, :], MIN); t.then_inc(sem_dve, 1)
                t = vector.tensor_tensor(mid3[:, :, :], pmn[:, 0:R, :], tt[:, :, :], MAX); t.then_inc(sem_dve, 1)
                t = vector.tensor_tensor(mlo[:, :, :], lo3[:, :, 0:W + 1], lo3[:, :, 1:WP], MAX); t.then_inc(sem_dve, 1)
                t = vector.tensor_tensor(mhi[:, :, :], hi3[:, :, 0:W + 1], hi3[:, :, 1:WP], MIN); t.then_inc(sem_dve, 1)
                t = vector.tensor_tensor(qmn[:, :, :], mid3[:, :, 0:W + 1], mid3[:, :, 1:WP], MIN); t.then_inc(sem_dve, 1)
                t = vector.tensor_tensor(qmx[:, :, :], mid3[:, :, 0:W + 1], mid3[:, :, 1:WP], MAX); t.then_inc(sem_dve, 1)
                t = vector.tensor_tensor(A[:, :, :], mlo[:, :, 0:W], lo3[:, :, 2:WP], MAX); t.then_inc(sem_dve, 1)
                t = vector.tensor_tensor(C[:, :, :], mhi[:, :, 0:W], hi3[:, :, 2:WP], MIN); t.then_inc(sem_dve, 1)
                t = vector.tensor_tensor(u[:, :, :], qmx[:, :, 0:W], mid3[:, :, 2:WP], MIN); t.then_inc(sem_dve, 1)
                t = vector.tensor_tensor(B[:, :, :], qmn[:, :, 0:W], u[:, :, :], MAX); t.then_inc(sem_dve, 1)
                t = vector.tensor_tensor(fmn[:, :, :], A[:, :, :], B[:, :, :], MIN); t.then_inc(sem_dve, 1)
                t = vector.tensor_tensor(fmx[:, :, :], A[:, :, :], B[:, :, :], MAX); t.then_inc(sem_dve, 1)
                t = vector.tensor_tensor(v[:, :, :], fmx[:, :, :], C[:, :, :], MIN); t.then_inc(sem_dve, 1)
                if i >= 2:
                    vector.wait_ge(sem_out, 32 * (i - 1))
                t = vector.tensor_tensor(ov[:, :, :], fmn[:, :, :], v[:, :, :], MAX); t.then_inc(sem_dve, 1)
    return nc


_NC_CACHE = {}


def _get_nc():
    if "nc" not in _NC_CACHE:
        _NC_CACHE["nc"] = _build_nc()
    return _NC_CACHE["nc"]


def kernel(noised_image, cover_image):
    x16 = np.ascontiguousarray(noised_image).astype(np.float16)
    nc = _get_nc()
    per = x16.shape[0] // N_CORES  # 4 images per core
    in_maps = []
    for c in range(N_CORES):
        shard = x16[c * per:(c + 1) * per].reshape(N_CH, H, W)
        padded = np.pad(shard, ((0, 0), (1, 1), (1, 1)), mode='edge')
        in_maps.append({"x": np.ascontiguousarray(padded)})
    res = bass_utils.run_bass_kernel_spmd(nc, in_maps, core_ids=list(range(N_CORES)))
    blurred = np.stack([r["y"].reshape(per, 3, H, W) for r in res.results])
    blurred = blurred.reshape(noised_image.shape).astype(np.float32)
    return (blurred, cover_image)


# revision 5
# speedup vs baseline: 3.3013x; 1.3259x over previous
"""3x3 median blur, fp16, R=8, 15-op network via row-adjacent op merging.

Same algorithm as kernel.py, but same-ALU op pairs are merged by laying the
operands out in adjacent row blocks of shared buffers (row-sliced writes are
fully dense, so the DVE fast mode is preserved):
  VHM [3R,514] = [lo3 | mid3 | hi3]
  - one MAX over VHM[0:2R] computes mlo and qmx together
  - one MIN over VHM[R:3R] computes qmn and mhi together
  AF [2R,512] = [A | fmx], BC [2R,512] = [B | C]
  - one MIN(AF, BC) computes fmn and v together
18 -> 15 ops/strip; semaphore thresholds rescaled to DOPS=15.
"""
import sys
sys.path.insert(0, '/opt/trn_rl_repo')
from contextlib import ExitStack
import numpy as np

import concourse.bass as bass
import concourse.mybir as mybir
import bass_rust
from concourse import bass_utils

F16 = mybir.dt.float16
MIN = mybir.AluOpType.min
MAX = mybir.AluOpType.max

N_CORES = 8
N_CH = 12
N_STRIPS = 6
H = W = 512
HP = WP = 514
R = 8


def _mk_ap(base, dims, offset):
    c = base.copy()
    c.ap = bass_rust.VecI64Pair(dims)
    c.offset = offset
    return c


def _build_nc(reps=1):
    nc = bass.Bass("TRN2")
    x = nc.dram_tensor("x", [N_CH, HP, WP], F16, kind="ExternalInput")
    y = nc.dram_tensor("y", [N_CH, W, W], F16, kind="ExternalOutput")
    DOPS = 15
    with ExitStack() as ctx:
        xs = [ctx.enter_context(nc.sbuf_tensor(f"xs{i}", [128, R + 2, WP], F16)) for i in range(2)]
        out = [ctx.enter_context(nc.sbuf_tensor(f"outb{i}", [128, R, W], F16)) for i in range(2)]
        pmn = ctx.enter_context(nc.sbuf_tensor("pmn", [128, R, WP], F16))
        pmx = ctx.enter_context(nc.sbuf_tensor("pmx", [128, R, WP], F16))
        tt = ctx.enter_context(nc.sbuf_tensor("tt", [128, R, WP], F16))
        VHM = ctx.enter_context(nc.sbuf_tensor("VHM", [128, 3 * R, WP], F16))   # lo3|mid3|hi3
        MQ = ctx.enter_context(nc.sbuf_tensor("MQ", [128, 2 * R, W], F16))      # mlo|qmx
        NQ = ctx.enter_context(nc.sbuf_tensor("NQ", [128, 2 * R, W], F16))      # qmn|mhi
        AF = ctx.enter_context(nc.sbuf_tensor("AF", [128, 2 * R, W], F16))      # A|fmx
        BC = ctx.enter_context(nc.sbuf_tensor("BC", [128, 2 * R, W], F16))      # B|C
        FV = ctx.enter_context(nc.sbuf_tensor("FV", [128, 2 * R, W], F16))      # fmn|v
        u = ctx.enter_context(nc.sbuf_tensor("u", [128, R, W], F16))

        sem_in = ctx.enter_context(nc.semaphore())
        sem_out = ctx.enter_context(nc.semaphore())
        sem_dve = ctx.enter_context(nc.semaphore())

        block = ctx.enter_context(nc.Block())
        n_strips = N_STRIPS * reps

        @block.sync
        def _(sync):
            for i in range(n_strips):
                s = i % N_STRIPS
                if i >= 2:
                    sync.wait_ge(sem_dve, DOPS * (i - 2) + 5)
                for h in range(2):
                    pl = 2 * s + h
                    src = _mk_ap(x[0], [[R * WP, 64], [WP, R + 2], [1, WP]],
                                 pl * HP * WP)
                    sync.dma_start(xs[i % 2][64 * h:64 * (h + 1), :, :], src).then_inc(sem_in, 16)
                if i >= 1:
                    oi = i - 1
                    sync.wait_ge(sem_dve, DOPS * (oi + 1))
                    os_ = oi % N_STRIPS
                    for h in range(2):
                        pl = 2 * os_ + h
                        dst = _mk_ap(y[0], [[R * W, 64], [W, R], [1, W]],
                                     pl * W * W)
                        sync.dma_start(dst, out[oi % 2][64 * h:64 * (h + 1), :, :]).then_inc(sem_out, 16)
            oi = n_strips - 1
            sync.wait_ge(sem_dve, DOPS * (oi + 1))
            os_ = oi % N_STRIPS
            for h in range(2):
                pl = 2 * os_ + h
                dst = _mk_ap(y[0], [[R * W, 64], [W, R], [1, W]], pl * W * W)
                sync.dma_start(dst, out[oi % 2][64 * h:64 * (h + 1), :, :]).then_inc(sem_out, 16)

        @block.vector
        def _(vector):
            R2, R3 = 2 * R, 3 * R
            for i in range(n_strips):
                xv = xs[i % 2]
                ov = out[i % 2]
                vector.wait_ge(sem_in, 32 * (i + 1))
                vector.tensor_tensor(pmn[:, :, :], xv[:, 0:R, :], xv[:, 1:R + 1, :], MIN)            # 1
                vector.tensor_tensor(pmx[:, :, :], xv[:, 0:R, :], xv[:, 1:R + 1, :], MAX)            # 2
                vector.tensor_tensor(VHM[:, 0:R, :], pmn[:, :, :], xv[:, 2:R + 2, :], MIN)           # 3 lo3
                vector.tensor_tensor(VHM[:, R2:R3, :], pmx[:, :, :], xv[:, 2:R + 2, :], MAX)         # 4 hi3
                t = vector.tensor_tensor(tt[:, :, :], pmx[:, :, :], xv[:, 2:R + 2, :], MIN); t.then_inc(sem_dve, 5)  # 5
                vector.tensor_tensor(VHM[:, R:R2, :], pmn[:, :, :], tt[:, :, :], MAX)                # 6 mid3
                vector.tensor_tensor(MQ[:, :, :], VHM[:, 0:R2, 0:W], VHM[:, 0:R2, 1:W + 1], MAX)     # 7 mlo|qmx
                vector.tensor_tensor(NQ[:, :, :], VHM[:, R:R3, 0:W], VHM[:, R:R3, 1:W + 1], MIN)     # 8 qmn|mhi
                vector.tensor_tensor(AF[:, 0:R, :], MQ[:, 0:R, :], VHM[:, 0:R, 2:WP], MAX)           # 9 A
                vector.tensor_tensor(u[:, :, :], MQ[:, R:R2, :], VHM[:, R:R2, 2:WP], MIN)            # 10 u
                vector.tensor_tensor(BC[:, 0:R, :], NQ[:, 0:R, :], u[:, :, :], MAX)                  # 11 B
                vector.tensor_tensor(BC[:, R:R2, :], NQ[:, R:R2, :], VHM[:, R2:R3, 2:WP], MIN)       # 12 C
                vector.tensor_tensor(AF[:, R:R2, :], AF[:, 0:R, :], BC[:, 0:R, :], MAX)              # 13 fmx
                vector.tensor_tensor(FV[:, :, :], AF[:, :, :], BC[:, :, :], MIN)                     # 14 fmn|v
                if i >= 2:
                    vector.wait_ge(sem_out, 32 * (i - 1))
                t = vector.tensor_tensor(ov[:, :, :], FV[:, 0:R, :], FV[:, R:R2, :], MAX); t.then_inc(sem_dve, 10)   # 15
    return nc


_NC_CACHE = {}


def _get_nc():
    if "nc" not in _NC_CACHE:
        _NC_CACHE["nc"] = _build_nc()
    return _NC_CACHE["nc"]


def kernel(noised_image, cover_image):
    x16 = np.ascontiguousarray(noised_image).astype(np.float16)
    nc = _get_nc()
    per = x16.shape[0] // N_CORES
    in_maps = []
    for c in range(N_CORES):
        shard = x16[c * per:(c + 1) * per].reshape(N_CH, H, W)
        padded = np.pad(shard, ((0, 0), (1, 1), (1, 1)), mode='edge')
        in_maps.append({"x": np.ascontiguousarray(padded)})
    res = bass_utils.run_bass_kernel_spmd(nc, in_maps, core_ids=list(range(N_CORES)))
    blurred = np.stack([r["y"].reshape(per, 3, H, W) for r in res.results])
    blurred = blurred.reshape(noised_image.shape).astype(np.float32)
    return (blurred, cover_image)
